# revision 8
# baseline (speedup 1.0000x reference)
"""Trainium2 Bass kernel for nn_CB_RNN_tiedcell (H=24, IN=8, B=1048576).

Math
----
reference(x, W, P, ...) computes, per batch column b:
    z_t = dt*sig(K@r + P_z@x_b + b_z)            (K, P_z, r, biases are batch-constant)
    v   = (1-z_t)*v0 + dt*(W@(U*X*r) + P@x_b + b_v)
All (24,1) state math (r, X, U, Ucap, clamp, K@r, W@u) is batch-constant and
precomputed on the host.  With s = sig(-(P_z@x_b + zpre)) = 1 - sig(+...):
    v[:,b] = dt*P@x_b + cv + dtv0 * s[:,b]
where cv = dt*(W@u + b_v) + (1-dt)*v0 and dtv0 = dt*v0.
When v0 == 0 (the shipped inputs), the whole sigmoid path vanishes.

Kernel layout (per core, data-parallel shard of B/8 = 131072 batches)
---------------------------------------------------------------------
Batch-major compute via a block-diagonal stationary trick: the PE stationary
operand is a [128, 128] tile of x holding S=16 independent 8-row sub-chunks
(sub-chunk s on partitions 8s..8s+7); the moving operand is a constant
block-diagonal weight matrix [128, S*24] with w_v^T (8,24) on the diagonal
blocks.  One matmul then yields a [128 (batch), S*24] PSUM tile = batch-major
outputs for S*128 batches, so one (expensive, ~107ns) f32 weight load covers
128*Q batches (Q matmuls per load... actually one load per matmul but M=128
columns amortized over S=16 sub-chunks: 2048 batches per LDW+MM pair).
A single fused DVE pass (scalar_tensor_tensor) adds the bias and moves
PSUM -> SBUF staging laid out so the output DMA writes (B_c, 24) DRAM rows
in contiguous >=1.5KB per-partition chunks.
"""

import numpy as np

H = 24
IN = 8
NCORES = 8
B_FULL = 1048576
F32 = None  # set lazily (mybir import) so numpy-only host code can be tested


def _np_softplus(x):
    x = np.asarray(x, np.float32)
    return np.logaddexp(np.float32(0.0), x).astype(np.float32)


def _np_sigmoid(x):
    x = np.asarray(x, np.float32)
    return (np.float32(1.0) / (np.float32(1.0) + np.exp(-x))).astype(np.float32)


def host_precompute(W, P, b_v, b_z, e, e_p, c_x, c_u, c_U, v0, X0, U0):
    """All (24,1)/(24,24) batch-constant math, in float32 mirroring the ref."""
    dt = np.float32(0.1)
    delta_t = np.float32(1.0)
    z_min, z_max = np.float32(0.001), np.float32(0.1)
    sp, sig = _np_softplus, _np_sigmoid

    W = np.asarray(W, np.float32)
    P = np.asarray(P, np.float32)
    b_v = np.asarray(b_v, np.float32).reshape(H, 1)
    b_z = np.asarray(b_z, np.float32).reshape(H, 1)
    v0 = np.asarray(v0, np.float32).reshape(H, 1)
    X0 = np.asarray(X0, np.float32).reshape(H, 1)
    U0 = np.asarray(U0, np.float32).reshape(H, 1)
    c_x = np.asarray(c_x, np.float32).reshape(H, 1)
    c_u = np.asarray(c_u, np.float32).reshape(H, 1)
    c_U = np.asarray(c_U, np.float32).reshape(H, 1)

    K = sp(np.float32(e).reshape(())) * sp(W)        # (H,H)
    P_z = sp(np.float32(e_p).reshape(())) * sp(P)    # (H,IN)

    r = sig(v0)                                      # (H,1)
    z_x = z_min + (z_max - z_min) * sig(c_x)
    X = z_x + (np.float32(1.0) - z_x) * X0 - delta_t * U0 * X0 * r
    z_u = z_min + (z_max - z_min) * sig(c_u)
    Ucap = np.float32(0.9) * sig(c_U)
    U = Ucap * z_u + (np.float32(1.0) - z_u) * U0 + delta_t * Ucap * (np.float32(1.0) - U0) * r
    U_c = np.clip(U, Ucap, np.float32(1.0))          # (H,1), batch-constant

    zpre = (K @ r + b_z).astype(np.float32)          # (H,1)
    u_vec = (U_c * X * r).astype(np.float32)         # (H,1)
    bias_v = (W @ u_vec + b_v).astype(np.float32)    # (H,1)

    w_v = (dt * P).T.astype(np.float32).copy()       # (IN,H)
    cv = (dt * bias_v + (np.float32(1.0) - dt) * v0).reshape(H).astype(np.float32)
    w_z = (-P_z).T.astype(np.float32).copy()         # (IN,H)
    cz = (-zpre).reshape(H).astype(np.float32)
    dtv0 = (dt * v0).reshape(H).astype(np.float32)
    return w_v, cv, w_z, cz, dtv0


def _block_diag(w, S):
    """w (IN,H) -> [128, S*H] with w on diagonal blocks of 8 rows x 24 cols."""
    out = np.zeros((128, S * H), np.float32)
    for s in range(S):
        out[8 * s : 8 * s + IN, H * s : H * s + H] = w
    return out


def build_program(B_c, Q, full_path):
    """Build the per-core Bass program.

    B_c: batches per core.  S=16 sub-chunks, supertile = S*128*Q batches.
    Q: batches per partition per matmul (also out-DMA chunk = 96*Q bytes).
    full_path: include the sigmoid correction term (needed iff v0 != 0).
    """
    import concourse.bass as bass
    import concourse.bacc as bacc
    import concourse.tile as tile
    from concourse import mybir

    S = 16
    WFREE = 128 * Q          # x-tile free elements per sub-chunk
    SUPER = S * WFREE        # batches per supertile
    assert B_c % SUPER == 0, (B_c, SUPER)
    NT = B_c // SUPER
    N = S * H                # matmul free dim = 384
    f32 = mybir.dt.float32

    nc = bacc.Bacc()
    x_in = nc.declare_dram_parameter("xs", [IN, B_c], f32, isOutput=False)
    wblk_in = nc.declare_dram_parameter("wblk", [128, N], f32, isOutput=False)
    cvec_in = nc.declare_dram_parameter("cvec", [1, N], f32, isOutput=False)
    if full_path:
        wblkz_in = nc.declare_dram_parameter("wblkz", [128, N], f32, isOutput=False)
        czvec_in = nc.declare_dram_parameter("czvec", [1, N], f32, isOutput=False)
        dvvec_in = nc.declare_dram_parameter("dvvec", [1, N], f32, isOutput=False)
    out_ext = nc.declare_dram_parameter("out", [B_c, H], f32, isOutput=True)

    AT = mybir.AluOpType
    with tile.TileContext(nc) as tc:
        with (
            tc.tile_pool(name="singles", bufs=1) as singles,
            tc.tile_pool(name="xp", bufs=2) as xp,
            tc.tile_pool(name="op", bufs=2) as op,
            tc.tile_pool(name="ps", bufs=4, space="PSUM") as psp,
            tc.tile_pool(name="sp", bufs=4) as sbp,
        ):
            wblk_sb = singles.tile([128, N], f32)
            nc.sync.dma_start(out=wblk_sb, in_=wblk_in[:, :])
            cv_rep = singles.tile([128, N], f32)
            nc.gpsimd.dma_start(out=cv_rep, in_=cvec_in[:, :].to_broadcast([128, N]))
            if full_path:
                wblkz_sb = singles.tile([128, N], f32)
                nc.sync.dma_start(out=wblkz_sb, in_=wblkz_in[:, :])
                cz_rep = singles.tile([128, N], f32)
                nc.gpsimd.dma_start(out=cz_rep, in_=czvec_in[:, :].to_broadcast([128, N]))
                dv_rep = singles.tile([128, N], f32)
                nc.gpsimd.dma_start(out=dv_rep, in_=dvvec_in[:, :].to_broadcast([128, N]))

            for T in range(NT):
                # ---- x load: partition 8s+k <- x[k, T*SUPER + s*WFREE ...] ----
                xt = xp.tile([128, WFREE], f32)
                src = x_in[:, :].rearrange(
                    "k (t s w) -> t s k w", s=S, w=WFREE)[T]
                nc.sync.dma_start(out=xt[:, :], in_=src)

                out_sb = op.tile([128, S * Q * H], f32)
                for j in range(Q):
                    # lhsT: [128, 128], free stride Q, offset j (batch m*Q+j)
                    lhsT = xt.rearrange("p (m q) -> p m q", q=Q)[:, :, j]
                    ps = psp.tile([128, N], f32)
                    nc.tensor.matmul(ps, lhsT, wblk_sb, start=True, stop=True)
                    o_v = out_sb.rearrange("p (s q h) -> p s q h", s=S, h=H)[:, :, j, :]
                    p_v = ps.rearrange("p (s h) -> p s h", h=H)
                    c_v3 = cv_rep.rearrange("p (s h) -> p s h", h=H)
                    if not full_path:
                        # out = ps + cv  (fused copy+bias, one DVE pass)
                        nc.vector.scalar_tensor_tensor(
                            out=o_v, in0=p_v, scalar=1.0, in1=c_v3,
                            op0=AT.mult, op1=AT.add,
                        )
                    else:
                        psz = psp.tile([128, N], f32)
                        nc.tensor.matmul(psz, lhsT, wblkz_sb, start=True, stop=True)
                        pz_v = psz.rearrange("p (s h) -> p s h", h=H)
                        cz_v = cz_rep.rearrange("p (s h) -> p s h", h=H)
                        dv_v = dv_rep.rearrange("p (s h) -> p s h", h=H)
                        zb = sbp.tile([128, N], f32)
                        zb_v = zb.rearrange("p (s h) -> p s h", h=H)
                        # zb = psz + cz
                        nc.vector.scalar_tensor_tensor(
                            out=zb_v, in0=pz_v, scalar=1.0, in1=cz_v,
                            op0=AT.mult, op1=AT.add,
                        )
                        # s = sig(zb)
                        sg = sbp.tile([128, N], f32)
                        nc.scalar.activation(
                            out=sg, in_=zb, func=mybir.ActivationFunctionType.Sigmoid,
                        )
                        sg_v = sg.rearrange("p (s h) -> p s h", h=H)
                        # t = sg * dtv0 + cv   (fused, one DVE pass)
                        tt = sbp.tile([128, N], f32)
                        tt_v = tt.rearrange("p (s h) -> p s h", h=H)
                        nc.vector.tensor_tensor(
                            out=tt_v, in0=sg_v, in1=dv_v, op=AT.mult,
                        )
                        # out = (tt + cv) + ps ... need 2 passes: u = tt+cv; out = u+ps
                        nc.vector.scalar_tensor_tensor(
                            out=tt_v, in0=tt_v, scalar=1.0, in1=c_v3,
                            op0=AT.mult, op1=AT.add,
                        )
                        nc.vector.scalar_tensor_tensor(
                            out=o_v, in0=p_v, scalar=1.0, in1=tt_v,
                            op0=AT.mult, op1=AT.add,
                        )

                # ---- out DMA: batch beta = T*SUPER + s*WFREE + m*Q + j ----
                dst_o = out_ext[:, :].rearrange(
                    "(t s m q) h -> t m s (q h)", s=S, m=128, q=Q)[T]
                src_o = out_sb.rearrange("p (s f) -> p s f", s=S)
                nc.sync.dma_start(out=dst_o, in_=src_o)
    nc.compile()  # bacc legalization: wait-splitting, event sems, table loads
    return nc


def _run(nc, in_maps, core_ids, trace=False):
    from concourse.bass_utils import run_bass_kernel_spmd
    return run_bass_kernel_spmd(nc, in_maps, core_ids, trace=trace)


def kernel(x, W, P, b_v, b_z, e, e_p, c_x, c_u, c_U, v0, X0, U0,
           _trace=False, _Q=32):
    x = np.ascontiguousarray(np.asarray(x, np.float32))
    assert x.shape == (IN, B_FULL), x.shape
    w_v, cv, w_z, cz, dtv0 = host_precompute(
        W, P, b_v, b_z, e, e_p, c_x, c_u, c_U, v0, X0, U0)
    full_path = bool(np.any(dtv0 != 0))

    S = 16
    B_c = B_FULL // NCORES
    nc = build_program(B_c, _Q, full_path)

    wblk = _block_diag(w_v, S)
    cvec = np.tile(cv, S).reshape(1, S * H).astype(np.float32)
    base = {"wblk": wblk, "cvec": cvec}
    if full_path:
        base["wblkz"] = _block_diag(w_z, S)
        base["czvec"] = np.tile(cz, S).reshape(1, S * H).astype(np.float32)
        base["dvvec"] = np.tile(dtv0, S).reshape(1, S * H).astype(np.float32)

    core_ids = list(range(NCORES))
    in_maps = []
    for c in core_ids:
        m = dict(base)
        m["xs"] = np.ascontiguousarray(x[:, c * B_c : (c + 1) * B_c])
        in_maps.append(m)

    res = _run(nc, in_maps, core_ids, trace=_trace)
    out = np.concatenate([res.results[i]["out"] for i in range(NCORES)], axis=0)
    if _trace:
        kernel.last_exec_time_ns = res.exec_time_ns
        kernel.last_results = res
    return out
